# revision 1
# baseline (speedup 1.0000x reference)
"""Multi-head attention (B=2, S=2048, E=1024, H=16, D=64, causal) on 8 TRN2 cores.

Sharding: core c -> (batch b = c//4, head-group g = c%4).  Each core computes
QKV projections for its 4 heads on its batch, causal attention for those
heads, then the 4 cores of a batch AllGather their attention outputs and each
computes a 256-column slice of the output projection.  The AllGather and
projection are split per (512-query block, head-pair) and pipelined into the
attention loop so only the last block's AG+proj sits on the critical tail.

Everything on-chip flows in transposed layout so attention needs no P
transposes: Q^T/K^T are [d, s]; scores are computed as S^T = K^T.T @ Q^T in
[k, q] layout; softmax denominators come free from a ones-column appended to
V (PV matmul row 64 = column sums of P); causal masking is a GPSIMD
affine_select on the exp'd P tiles.  Attention matmuls run in float32r
(TF32-like, full PE rate for moving dims >= 256); the projection runs bf16.
"""

import numpy as np

B, S, E = 2, 2048, 1024
H, D = 16, 64
N_CORES = 8
GROUPS = [[0, 1, 2, 3], [4, 5, 6, 7]]
OSL = 256  # per-core head-column slice (4 heads * 64)

_CACHE = {}


def build_nc():
    import concourse.bass as bass
    import concourse.mybir as mybir
    import concourse.tile as tile
    from concourse import bacc
    from concourse.masks import make_identity

    f32 = mybir.dt.float32
    f32r = mybir.dt.float32r
    bf16 = mybir.dt.bfloat16
    Exp = mybir.ActivationFunctionType.Exp
    MS = bass.MemorySpace

    nc = bacc.Bacc("TRN2", target_bir_lowering=False, debug=False,
                   num_devices=N_CORES)

    xd = nc.dram_tensor("x", [S, E], f32, kind="ExternalInput")
    wqd = nc.dram_tensor("wq", [OSL, E], f32, kind="ExternalInput")
    wkd = nc.dram_tensor("wk", [OSL, E], f32, kind="ExternalInput")
    wvd = nc.dram_tensor("wv", [OSL, E], f32, kind="ExternalInput")
    wpd = nc.dram_tensor("wp", [OSL, E], f32, kind="ExternalInput")
    bpd = nc.dram_tensor("bp", [1, OSL], f32, kind="ExternalInput")
    outd = nc.dram_tensor("out", [S, OSL], f32, kind="ExternalOutput")

    with tile.TileContext(nc) as tc:
        with (
            tc.tile_pool(name="const", bufs=1) as constp,
            tc.tile_pool(name="wT", bufs=1) as wTp,
            tc.tile_pool(name="outsb", bufs=4) as outp,
            tc.tile_pool(name="dram", bufs=1, space="DRAM") as dram,
            tc.tile_pool(name="wrow", bufs=3) as wrowp,
            tc.tile_pool(name="xrow", bufs=8) as xrowp,
            tc.tile_pool(name="xT", bufs=9) as xTp,
            tc.tile_pool(name="qkT", bufs=1) as qkTp,
            tc.tile_pool(name="pT", bufs=4) as pTp,
            tc.tile_pool(name="nrm", bufs=4) as nrmp,
            tc.tile_pool(name="attn", bufs=2) as attnp,
            tc.tile_pool(name="cT", bufs=12) as cTp,
            tc.tile_pool(name="tacc", bufs=2, space=MS.PSUM) as taccp,
            tc.tile_pool(name="scor", bufs=2, space=MS.PSUM) as scorp,
            tc.tile_pool(name="pvps", bufs=2, space=MS.PSUM) as pvpsp,
        ):
            ident = constp.tile([128, 128], f32, name="ident")
            make_identity(nc, ident[:])
            ident_r = constp.tile([128, 128], f32r, name="ident_r")
            nc.vector.tensor_copy(ident_r[:], ident[:])

            ones64_32 = constp.tile([1, 64], f32, name="ones64_32")
            nc.vector.memset(ones64_32[:], 1.0)
            ones33_32 = constp.tile([33, 64], f32, name="ones33_32")
            nc.vector.memset(ones33_32[:], 1.0)
            ones33 = constp.tile([33, 64], f32r, name="ones33")
            nc.vector.tensor_copy(ones33[:], ones33_32[:])
            ones128_32 = constp.tile([1, 128], f32, name="ones128_32")
            nc.vector.memset(ones128_32[:], 1.0)
            ones128_bf = constp.tile([1, 128], bf16, name="ones128_bf")
            nc.vector.tensor_copy(ones128_bf[:], ones128_32[:])
            onescol4 = constp.tile([128, 4], f32, name="onescol4")
            nc.vector.memset(onescol4[:], 1.0)

            bp_sb = constp.tile([1, OSL], f32, name="bp_sb")
            nc.sync.dma_start(bp_sb[:], bpd[:])
            bp_bf = constp.tile([1, OSL], bf16, name="bp_bf")
            nc.vector.tensor_copy(bp_bf[:], bp_sb[:])

            # batched-reciprocal staging (rows 0 and 32 used)
            sums2 = constp.tile([33, 512], f32, name="sums2")
            nc.vector.memset(sums2[:], 1.0)
            recip2 = constp.tile([33, 512], f32r, name="recip2")

            # W^T tiles: [i_inner 128, i_tile 8, o 256] per projection
            wT = {}
            for nm in ("wq", "wk", "wv"):
                wT[nm] = wTp.tile([128, 8, OSL], f32r, name=f"wT_{nm}",
                                  tag=f"wT_{nm}")
            wT["wp"] = wTp.tile([128, 8, OSL], bf16, name="wT_wp", tag="wT_wp")

            ag_in = [[dram.tile([128, 512], bf16, name=f"ag_in{q}_{hp}",
                                tag=f"ag_in{q}_{hp}") for hp in range(2)]
                     for q in range(4)]
            ag_out = [[dram.tile([512, 512], bf16, name=f"ag_out{q}_{hp}",
                                 tag=f"ag_out{q}_{hp}") for hp in range(2)]
                      for q in range(4)]

            # ---- weight transposes: w rows [o 128, i 1024] -> wT [i, o]
            for nm, wdram in (("wq", wqd), ("wk", wkd), ("wv", wvd),
                              ("wp", wpd)):
                for osub in range(2):
                    wrow = wrowp.tile([128, E], f32r, tag="wrow", name="wrow")
                    nc.sync.dma_start(wrow[:],
                                      wdram[osub * 128:(osub + 1) * 128, :]
                                      .bitcast(f32r))
                    for ith in range(2):
                        tp = taccp.tile([128, 512], f32r, tag="tacc", name="tp")
                        for j in range(4):
                            it = ith * 4 + j
                            nc.tensor.transpose(
                                tp[:, j * 128:(j + 1) * 128],
                                wrow[:, it * 128:(it + 1) * 128],
                                ident_r[:])
                        nc.vector.tensor_copy(
                            wT[nm][:, ith * 4:(ith + 1) * 4,
                                   osub * 128:(osub + 1) * 128],
                            tp[:].rearrange("p (j c) -> p j c", j=4))

            # persistent QKV results
            qT = [qkTp.tile([128, 4, 512], f32r, name=f"qT{hp}",
                            tag=f"qT{hp}") for hp in range(2)]
            kT = [qkTp.tile([128, 4, 512], f32r, name=f"kT{hp}",
                            tag=f"kT{hp}") for hp in range(2)]
            vE = qkTp.tile([128, 16, 4, 65], f32r, name="vE", tag="vE")

            def load_xrows(ssb):
                rows = []
                for ss in range(4):
                    xr = xrowp.tile([128, E], f32r, tag="xrow", name="xr")
                    r0 = (ssb * 4 + ss) * 128
                    nc.sync.dma_start(xr[:], xd[r0:r0 + 128, :].bitcast(f32r))
                    rows.append(xr)
                return rows

            def emit_proj_ct(qsb, cts):
                for hp in range(2):
                    for g in range(4):
                        c = cTp.tile([128, 512], bf16, tag="cT", name="cT")
                        nc.sync.dma_start(
                            c[:], ag_out[qsb][hp][g * 128:(g + 1) * 128, :])
                        cts[(g, hp)] = c

            def emit_proj_po(qsb, cts, qq):
                qblk = qsb * 4 + qq
                po = taccp.tile([128, 512], f32, tag="tacc", name="po")
                first = True
                for hp in range(2):
                    for g in range(4):
                        et = 2 * g + hp
                        nc.tensor.matmul(
                            po[:, 0:OSL],
                            cts[(g, hp)][:, qq * 128:(qq + 1) * 128],
                            wT["wp"][:, et, :],
                            start=first, stop=False)
                        first = False
                nc.tensor.matmul(po[:, 0:OSL], ones128_bf[:], bp_bf[:],
                                 start=False, stop=True)
                osb = outp.tile([128, OSL], f32, tag="outsb", name="osb")
                nc.vector.tensor_copy(osb[:], po[:, 0:OSL])
                nc.sync.dma_start(outd[qblk * 128:(qblk + 1) * 128, :],
                                  osb[:])

            def build_proj_units(qsb):
                cts = {}
                units = [lambda q=qsb, c=cts: emit_proj_ct(q, c)]
                for qq in range(4):
                    units.append(lambda q=qsb, c=cts, k=qq: emit_proj_po(q, c, k))
                return units

            def build_qkv_units(ssb, xrows):
                """Transposes + Q/K/V accumulation for one 512-row x block,
                as a list of closures (filler units)."""
                xTs = [None] * 8

                def transp_unit(it):
                    def u():
                        tp = taccp.tile([128, 512], f32r, tag="tacc", name="tp")
                        for ss in range(4):
                            nc.tensor.transpose(
                                tp[:, ss * 128:(ss + 1) * 128],
                                xrows[ss][:, it * 128:(it + 1) * 128],
                                ident_r[:])
                        xT = xTp.tile([128, 512], f32r, tag="xT", name="xT")
                        nc.vector.tensor_copy(xT[:], tp[:])
                        xTs[it] = xT
                    return u

                def qk_unit(nm, dst, osub):
                    def u():
                        acc = taccp.tile([128, 512], f32, tag="tacc", name="acc")
                        for it in range(8):
                            nc.tensor.matmul(
                                acc[:],
                                wT[nm][:, it, osub * 128:(osub + 1) * 128],
                                xTs[it][:],
                                start=(it == 0), stop=(it == 7))
                        nc.vector.tensor_copy(dst[osub][:, ssb, :], acc[:])
                    return u

                def v_unit(ss):
                    def u():
                        acc = taccp.tile([128, 512], f32, tag="tacc", name="acc")
                        for it in range(8):
                            nc.tensor.matmul(
                                acc[:, 0:OSL],
                                xTs[it][:, ss * 128:(ss + 1) * 128],
                                wT["wv"][:, it, :],
                                start=(it == 0), stop=(it == 7))
                        sblk = ssb * 4 + ss
                        nc.vector.tensor_copy(
                            vE[:, sblk, :, 0:64],
                            acc[:, 0:OSL].rearrange("p (h d) -> p h d", h=4))
                        nc.vector.tensor_copy(vE[:, sblk, :, 64], onescol4[:])
                    return u

                units = [transp_unit(it) for it in range(8)]
                units += [qk_unit("wq", qT, 0), qk_unit("wq", qT, 1),
                          qk_unit("wk", kT, 0), qk_unit("wk", kT, 1)]
                units += [v_unit(ss) for ss in range(4)]
                return units

            from collections import deque

            # prologue: x block 0 QKV emitted directly
            xrows0 = load_xrows(0)
            for u in build_qkv_units(0, xrows0):
                u()

            xrows_next = load_xrows(1)
            pending_norm = None   # carried (front, back) of previous qsb hp1
            for ssb in range(4):
                qsb = ssb
                nk = 4 * qsb + 4

                filler = deque()
                if pending_norm is not None:
                    filler.append(pending_norm[0])
                if ssb < 3:
                    qkv_units = build_qkv_units(ssb + 1, xrows_next)
                    filler.extend(qkv_units[:3])
                    if pending_norm is not None:
                        filler.append(pending_norm[1])
                    filler.extend(qkv_units[3:])
                elif pending_norm is not None:
                    filler.append(pending_norm[1])
                pending_norm = None
                proj_units = deque(build_proj_units(ssb - 1)) if ssb >= 1 \
                    else deque()

                def make_norm(qsb, hp, pv):
                    # normalize both heads (batched reciprocal at rows
                    # {0, 32}), stash to attnout, then AllGather.  Split so
                    # the DVE-only reciprocal chain (front) can be emitted
                    # ahead of PE work while the broadcast matmul + mul
                    # (back) lands later, once the chain has drained.
                    state = {}

                    def front():
                        for a in range(2):
                            nc.vector.tensor_copy(sums2[32 * a:32 * a + 1, :],
                                                  pv[a][64:65, :])
                        with nc.allow_low_precision(reason="softmax denom"):
                            nc.vector.reciprocal(recip2[:], sums2[:])
                        for a in range(2):
                            unnorm = nrmp.tile([64, 512], f32, tag="unnorm",
                                               name="unnorm")
                            nc.vector.tensor_copy(unnorm[:], pv[a][0:64, :])
                            state[a] = unnorm

                    def back():
                        attnout = attnp.tile([128, 512], bf16, tag="attnout",
                                             name="attnout")
                        for a in range(2):
                            pb = taccp.tile([128, 512], f32, tag="tacc",
                                            name="pb")
                            nc.tensor.matmul(pb[0:64, :],
                                             ones33[32 * a:32 * a + 1, :],
                                             recip2[32 * a:32 * a + 1, :])
                            nc.vector.tensor_tensor(
                                out=attnout[a * 64:(a + 1) * 64, :],
                                in0=pb[0:64, :], in1=state[a][:],
                                op=mybir.AluOpType.mult)
                        nc.scalar.dma_start(ag_in[qsb][hp][:], attnout[:])
                        nc.gpsimd.collective_compute(
                            "AllGather", mybir.AluOpType.bypass,
                            replica_groups=GROUPS,
                            ins=[ag_in[qsb][hp].opt()],
                            outs=[ag_out[qsb][hp].opt()])
                    return front, back

                inner_norm = None  # hp0's (front, back), fired during hp1
                for hp in range(2):
                    pv = [pvpsp.tile([128, 512], f32, tag="pvps", name="pv")
                          for _ in range(2)]

                    def emit_pv(kblk, pt, q0):
                        for a in range(2):
                            h = 2 * hp + a
                            nc.tensor.matmul(
                                pv[a][0:65, q0:512],
                                vE[:, kblk, h, :],
                                pt[:, a, q0:512],
                                start=(kblk == 0), stop=(kblk == nk - 1))

                    # software pipeline: PV for kblk-1 is emitted after the
                    # scores matmuls of kblk, so the PE never waits on the
                    # ACT exp + GPSIMD mask latency of the current kblk.
                    # Diagonal-band tiles (r >= 0) only compute q >= 128*r.
                    pending = None  # (kblk, pt, q0)
                    for kblk in range(nk):
                        sk, off = kblk // 4, (kblk % 4) * 128
                        r = kblk - 4 * qsb
                        q0 = 128 * r if r > 0 else 0
                        sc = scorp.tile([128, 2, 512], f32, tag="scor",
                                        name="sc")
                        for a in range(2):
                            nc.tensor.matmul(
                                sc[:, a, q0:512],
                                kT[hp][a * 64:(a + 1) * 64, sk, off:off + 128],
                                qT[hp][a * 64:(a + 1) * 64, qsb, q0:512])
                        pt = pTp.tile([128, 2, 512], f32r, tag="pT", name="pt")
                        nc.scalar.activation(pt[:, :, q0:512],
                                             sc[:, :, q0:512], Exp,
                                             scale=0.125)
                        if r >= 0:  # causal mask on the diagonal triangle
                            for a in range(2):
                                nc.gpsimd.affine_select(
                                    out=pt[:, a, q0:q0 + 128],
                                    in_=pt[:, a, q0:q0 + 128],
                                    compare_op=mybir.AluOpType.is_ge,
                                    fill=0.0, base=0,
                                    pattern=[[1, 128]],
                                    channel_multiplier=-1)
                        if pending is not None:
                            emit_pv(*pending)
                        pending = (kblk, pt, q0)
                        if inner_norm is not None:
                            if kblk == 0:
                                inner_norm[0]()
                            elif kblk == (4 if nk > 4 else 2):
                                inner_norm[1]()
                                inner_norm = None
                        if hp == 1 and kblk >= 3 and proj_units:
                            proj_units.popleft()()
                        elif filler:
                            filler.popleft()()
                    emit_pv(*pending)
                    if hp == 0:
                        inner_norm = make_norm(qsb, 0, pv)
                    else:
                        pending_norm = make_norm(qsb, 1, pv)

                while filler:
                    filler.popleft()()
                while proj_units:
                    proj_units.popleft()()
                if ssb < 3:
                    xrows_next = load_xrows(ssb + 2) if ssb < 2 else None

            pending_norm[0]()
            pending_norm[1]()
            cts3 = {}
            emit_proj_ct(3, cts3)
            for qq in range(4):
                emit_proj_po(3, cts3, qq)

    nc.compile()
    return nc


def _get_nc():
    if "nc" not in _CACHE:
        _CACHE["nc"] = build_nc()
    return _CACHE["nc"]


def _in_maps(x, wq, wk, wv, wp, bp):
    x = np.asarray(x, dtype=np.float32)
    wq = np.asarray(wq, dtype=np.float32)
    wk = np.asarray(wk, dtype=np.float32)
    wv = np.asarray(wv, dtype=np.float32)
    wp = np.asarray(wp, dtype=np.float32)
    bp = np.asarray(bp, dtype=np.float32)
    maps = []
    for c in range(N_CORES):
        b, g = divmod(c, 4)
        sl = slice(OSL * g, OSL * (g + 1))
        maps.append({
            "x": np.ascontiguousarray(x[b]),
            "wq": np.ascontiguousarray(wq[sl]),
            "wk": np.ascontiguousarray(wk[sl]),
            "wv": np.ascontiguousarray(wv[sl]),
            "wp": np.ascontiguousarray(wp[sl]),
            "bp": np.ascontiguousarray(bp[sl].reshape(1, OSL)),
        })
    return maps


def _assemble(results):
    out = np.empty((B, S, E), dtype=np.float32)
    for c in range(N_CORES):
        b, g = divmod(c, 4)
        out[b, :, OSL * g:OSL * (g + 1)] = results[c]["out"]
    return out


def kernel(x, wq, wk, wv, wp, bp):
    from concourse.bass_utils import run_bass_kernel_spmd
    nc = _get_nc()
    res = run_bass_kernel_spmd(nc, _in_maps(x, wq, wk, wv, wp, bp),
                               core_ids=list(range(N_CORES)))
    return _assemble(res.results)


def run_traced(x, wq, wk, wv, wp, bp, **kw):
    """For test.py: run with NTFF tracing, return (output, BassKernelResults)."""
    from concourse.bass_utils import run_bass_kernel_spmd
    nc = _get_nc()
    res = run_bass_kernel_spmd(nc, _in_maps(x, wq, wk, wv, wp, bp),
                               core_ids=list(range(N_CORES)), trace=True, **kw)
    return _assemble(res.results), res

